# revision 12
# baseline (speedup 1.0000x reference)
"""CGC layer (gated graph conv message passing) on 8 trn2 NeuronCores.

Math (per edge e with sender s, receiver r):
    c    = [x[s], x[r], ef[e]]                  # [320]
    vals = softplus(c @ W_val.T + b_val)        # [128]
    gate = sigmoid (c @ W_mul.T + b_mul)        # [128]
    out[r] += vals * gate                       # segment-sum over receivers

Strategy (edge-parallel, receiver-sharded => no cross-core reduction):
  * Host: LPT-balance nodes into 392 blocks of 128 so every block has
    <= K*128 incident edges with K=16 (vs 18 for the naive contiguous
    partition); shard 49 blocks/core.  Pre-gather x[s]/x[r] rows into
    edge-aligned fp16 streams [128, E_pad]; edge features (+bias row)
    feature-major [65, E_pad]; the one-hot scatter selector is also
    prebuilt on the host ([128, E_pad], 1 col per edge) so no DVE work
    is spent building it on device.
  * Device per chunk of 128 edges: 3 fp16 matmuls (fused [val|mul]
    weights, mul half negated, N=256) accumulate [A|-B] in PSUM; ACT Exp
    (single natural_log_exp table set, forced via table-membership
    steering so Exp/Ln never thrash table loads) gives [t|u]; ACT
    Ln(bias=1) gives vals=softplus(A); DVE/GpSimd compute
    msg = vals/(1+u); PE scatter-adds via psum_out += sel.T @ msg.
  * Software pipelining: the scatter matmuls of block b are emitted after
    the main matmuls of block b+1 so the PE never stalls on the ACT->DVE
    msg chain.
"""

import heapq
import os
import sys

sys.path.insert(0, "/opt/trn_rl_repo")

import numpy as np

from concourse import bacc, bass, mybir, tile
from concourse.bass_utils import run_bass_kernel_spmd

N_CORES = 8
P = 128            # partition / chunk size
G = 4              # chunks per PSUM group
NODE_DIM = 128
EDGE_DIM = 64
F16 = mybir.dt.float16
F32 = mybir.dt.float32

PIPE = os.environ.get("CGC_PIPE", "1") == "1"      # software pipelining
DEPTH = int(os.environ.get("CGC_DEPTH", "2"))      # scatter delay (blocks)
TABLEFIX = os.environ.get("CGC_TABLEFIX", "1") == "1"
PPOOL_BUFS = int(os.environ.get("CGC_PPOOL_BUFS", "3"))
GATE_MODE = os.environ.get("CGC_GATE", "fused")    # fused | recip

# Constants from RECIPROCAL_APPROX_FAST: Chebyshev-minimax seed pair over the
# [-4.5,-4] interval that x*bitcast(~x) lands in; one inline NR pass gives
# <=0.18% relative error on 1/(1+u) -- far inside the 2e-2 gate.
_GATE_C0 = -0.23549792
_GATE_C1 = 2.0017324


def _register_fused_gate():
    """Register a custom DVE op computing out = recip(in0 + 1) * in1 in one
    Vector instruction (bitwise-NOT reciprocal seed + one Newton step + the
    final multiply), replacing the 3-instruction add/recip/mult gate chain.
    Additive registration via the documented dve_ops extension point; sha is
    computed locally the same way DveOp.compile() checks it."""
    import concourse.dve_ops as dv
    from concourse.dve_spec import AluOp, Bin, Spec, Src0, Src1, C0, C1, C2, lower
    from concourse.dve_uop import DveOpSpec

    name = "CGC_GATE_FUSED"
    for op in dv.OPS:
        if op.name == name:
            return op
    w = Src0 + C2
    nw = Bin(AluOp.BITWISE_NOT, w, w)
    y0 = nw * C0
    y1 = y0 * (C1 - w * y0)
    body = y1 * Src1

    def _ref(in0, in1, s0, s1, imm2):
        wv = in0.astype(np.float32) + np.float32(imm2)
        nwv = (~wv.view(np.int32)).view(np.float32)
        y0v = nwv * np.float32(s0)
        y1v = y0v * (np.float32(s1) - wv * y0v)
        return (y1v * in1).astype(np.float32)

    spec = Spec(body=body, reference=_ref)
    row = max(dv._SUB_OPCODE_FOR_NAME.values()) + 1
    assert row < 0x20, "no free custom-DVE opcode rows"
    dv._SUB_OPCODE_FOR_NAME[name] = row
    shas = {}
    for ver in ("v3", "v4"):
        uops = lower(spec, ver=ver)
        shas[ver] = DveOpSpec(name=name, opcode=row, uops=uops, rd1_en=True).sha(ver)
    op = dv.DveOp(name, spec, subdim=False, uops_sha=shas)
    dv.OPS.append(op)
    dv.CUSTOM_DVE_SPECS[name] = spec
    return op


# ----------------------------------------------------------------- host prep
def _balance_blocks(deg, n_blocks):
    """LPT bin-pack nodes into n_blocks blocks of <=P nodes, balancing the
    per-block edge counts. Returns blk_of[node], pos_in_blk[node], sums."""
    n = deg.shape[0]
    order = np.argsort(-deg, kind="stable")
    heap = [(0, b) for b in range(n_blocks)]
    heapq.heapify(heap)
    used = np.zeros(n_blocks, dtype=np.int64)
    sums = np.zeros(n_blocks, dtype=np.int64)
    blk_of = np.empty(n, dtype=np.int64)
    pos_in_blk = np.empty(n, dtype=np.int64)
    for nid in order:
        while True:
            _, b = heapq.heappop(heap)
            if used[b] < P:
                break
        blk_of[nid] = b
        pos_in_blk[nid] = used[b]
        used[b] += 1
        sums[b] += deg[nid]
        if used[b] < P:
            heapq.heappush(heap, (sums[b], b))
    return blk_of, pos_in_blk, sums


def _preprocess(x, edge_index, edge_ft, W_val, b_val, W_mul, b_mul):
    n_nodes = x.shape[0]
    snd = np.asarray(edge_index[0], dtype=np.int64)
    rcv = np.asarray(edge_index[1], dtype=np.int64)

    blocks_per_core = int(np.ceil(n_nodes / (N_CORES * P)))  # 49 for 50000
    n_blocks = N_CORES * blocks_per_core
    B = blocks_per_core

    deg = np.bincount(rcv, minlength=n_nodes)
    blk_of, pos_in_blk, sums = _balance_blocks(deg, n_blocks)
    k_chunks = int(np.ceil(sums.max() / P))
    k_chunks = max(G, int(np.ceil(k_chunks / G)) * G)
    K = k_chunks
    e_pad = B * K * P

    # edge -> (core, slot)
    eb = blk_of[rcv]
    eorder = np.argsort(eb, kind="stable")
    eb_s = eb[eorder]
    snd_s = snd[eorder]
    rcv_s = rcv[eorder]
    counts = np.bincount(eb_s, minlength=n_blocks)
    starts = np.zeros(n_blocks + 1, dtype=np.int64)
    np.cumsum(counts, out=starts[1:])
    within = np.arange(len(eb_s), dtype=np.int64) - starts[eb_s]
    core_of = eb_s // B
    slot = (eb_s % B) * (K * P) + within

    x16 = np.asarray(x, dtype=np.float32).astype(np.float16)

    xsT = np.zeros((N_CORES, NODE_DIM, e_pad), dtype=np.float16)
    xrT = np.zeros((N_CORES, NODE_DIM, e_pad), dtype=np.float16)
    xsT[core_of, :, slot] = x16[snd_s]
    xrT[core_of, :, slot] = x16[rcv_s]

    ef16 = np.asarray(edge_ft, dtype=np.float32).astype(np.float16)
    efT = np.zeros((N_CORES, EDGE_DIM + 1, e_pad), dtype=np.float16)
    efT[core_of, :EDGE_DIM, slot] = ef16[eorder]
    efT[:, EDGE_DIM, :] = np.float16(1.0)     # bias row

    # host-built one-hot scatter selector: SEL[p, chunk*P + j] = (rloc==j)
    chunk = slot // P
    epos = slot % P
    SEL = np.zeros((N_CORES, P, e_pad), dtype=np.float16)
    SEL[core_of, epos, chunk * P + pos_in_blk[rcv_s]] = np.float16(1.0)

    # weights: [val | mul] fused on N; mul half negated so one Exp pass
    # yields [e^A | e^-B].
    Wv = np.asarray(W_val, dtype=np.float32)
    Wm = -np.asarray(W_mul, dtype=np.float32)
    bv = np.asarray(b_val, dtype=np.float32)
    bm = -np.asarray(b_mul, dtype=np.float32)
    Wsnd = np.concatenate([Wv[:, :128].T, Wm[:, :128].T], axis=1).astype(np.float16)
    Wrcv = np.concatenate([Wv[:, 128:256].T, Wm[:, 128:256].T], axis=1).astype(
        np.float16
    )
    Wef = np.concatenate(
        [
            np.concatenate([Wv[:, 256:320].T, Wm[:, 256:320].T], axis=1),
            np.concatenate([bv, bm])[None, :],
        ],
        axis=0,
    ).astype(np.float16)

    # output row of node n = blk_of[n]*P + pos_in_blk[n] (blocks core-major)
    row_of_node = blk_of * P + pos_in_blk

    return dict(
        xsT=xsT, xrT=xrT, efT=efT, SEL=SEL,
        Wsnd=Wsnd, Wrcv=Wrcv, Wef=Wef,
        B=B, K=K, e_pad=e_pad, row_of_node=row_of_node,
    )


# ------------------------------------------------------------- device kernel
def _build_nc(B, K, e_pad):
    n_groups = K // G
    nc = bacc.Bacc("TRN2", target_bir_lowering=False, debug=False)

    xsT = nc.dram_tensor("xsT", [NODE_DIM, e_pad], F16, kind="ExternalInput")
    xrT = nc.dram_tensor("xrT", [NODE_DIM, e_pad], F16, kind="ExternalInput")
    efT = nc.dram_tensor("efT", [EDGE_DIM + 1, e_pad], F16, kind="ExternalInput")
    SEL = nc.dram_tensor("SEL", [P, e_pad], F16, kind="ExternalInput")
    Wsnd_d = nc.dram_tensor("Wsnd", [NODE_DIM, 256], F16, kind="ExternalInput")
    Wrcv_d = nc.dram_tensor("Wrcv", [NODE_DIM, 256], F16, kind="ExternalInput")
    Wef_d = nc.dram_tensor("Wef", [EDGE_DIM + 1, 256], F16, kind="ExternalInput")
    out_d = nc.dram_tensor("out", [B * P, NODE_DIM], F16, kind="ExternalOutput")

    with tile.TileContext(nc) as tc:
        with (
            tc.tile_pool(name="const", bufs=1) as cpool,
            tc.tile_pool(name="stream", bufs=2 + DEPTH) as spool,
            tc.tile_pool(name="blk", bufs=1 + DEPTH) as bpool,
            tc.tile_pool(name="psumAB", bufs=PPOOL_BUFS, space="PSUM") as ppool,
            tc.tile_pool(name="psumOut", bufs=2, space="PSUM") as opool,
        ):
            w_snd = cpool.tile([NODE_DIM, 256], F16)
            w_rcv = cpool.tile([NODE_DIM, 256], F16)
            w_ef = cpool.tile([EDGE_DIM + 1, 256], F16)
            nc.sync.dma_start(out=w_snd[:], in_=Wsnd_d[:])
            nc.sync.dma_start(out=w_rcv[:], in_=Wrcv_d[:])
            nc.sync.dma_start(out=w_ef[:], in_=Wef_d[:])

            gate_op = _register_fused_gate() if GATE_MODE == "fused" else None

            def compute_block(b):
                off = b * K * P
                xs_b = spool.tile([NODE_DIM, K * P], F16, tag="xs")
                xr_b = spool.tile([NODE_DIM, K * P], F16, tag="xr")
                ef_b = spool.tile([EDGE_DIM + 1, K * P], F16, tag="ef")
                sel = spool.tile([P, K, P], F16, tag="sel")
                nc.sync.dma_start(out=xs_b[:], in_=xsT[:, off:off + K * P])
                nc.sync.dma_start(out=xr_b[:], in_=xrT[:, off:off + K * P])
                nc.sync.dma_start(out=ef_b[:], in_=efT[:, off:off + K * P])
                nc.sync.dma_start(out=sel[:], in_=SEL[:, off:off + K * P])

                tu = bpool.tile([P, K, 256], F16, tag="tu")
                for g in range(n_groups):
                    pab = ppool.tile([P, G, 256], F32, tag="ab")
                    for q in range(G):
                        c = g * G + q
                        sl = slice(c * P, (c + 1) * P)
                        nc.tensor.matmul(
                            out=pab[:, q, :], lhsT=xs_b[:, sl], rhs=w_snd[:],
                            start=True, stop=False,
                        )
                        nc.tensor.matmul(
                            out=pab[:, q, :], lhsT=xr_b[:, sl], rhs=w_rcv[:],
                            start=False, stop=False,
                        )
                        nc.tensor.matmul(
                            out=pab[:, q, :], lhsT=ef_b[:, sl], rhs=w_ef[:],
                            start=False, stop=True,
                        )
                    gsl = slice(g * G, (g + 1) * G)
                    nc.scalar.activation(
                        out=tu[:, gsl, :], in_=pab[:],
                        func=mybir.ActivationFunctionType.Exp,
                    )

                # vals = ln(1 + t)   [softplus]; tiles are 2D so the fused
                # gate op sees a 1-free-dim src1 (TTSS shape keeps imm2)
                vals = bpool.tile([P, K * P], F16, tag="vals")
                nc.scalar.activation(
                    out=vals[:], in_=tu[:, :, 0:P],
                    func=mybir.ActivationFunctionType.Ln, bias=1.0,
                )
                # msg = vals / (1 + u)
                msg = bpool.tile([P, K * P], F16, tag="msg")
                if gate_op is not None:
                    nc.vector._custom_dve(
                        gate_op, out=msg[:], in0=tu[:, :, P:256], in1=vals[:],
                        s0=_GATE_C0, s1=_GATE_C1, imm2=1.0,
                    )
                else:
                    w32 = bpool.tile([P, K, P], F32, tag="w32")
                    g32 = bpool.tile([P, K, P], F32, tag="g32")
                    nc.vector.tensor_scalar_add(w32[:], tu[:, :, P:256], 1.0)
                    nc.vector.reciprocal_approx_fast(out=g32[:], in_=w32[:])
                    nc.vector.tensor_tensor(
                        out=msg[:], in0=vals[:], in1=g32[:],
                        op=mybir.AluOpType.mult,
                    )
                return sel, msg

            def scatter_block(sel, msg, b):
                pout = opool.tile([P, P], F32, tag="out")
                for c in range(K):
                    nc.tensor.matmul(
                        out=pout[:], lhsT=sel[:, c, :],
                        rhs=msg[:, c * P:(c + 1) * P],
                        start=(c == 0), stop=(c == K - 1),
                    )
                o_sb = bpool.tile([P, P], F16, tag="osb")
                nc.vector.tensor_copy(out=o_sb[:], in_=pout[:])
                nc.sync.dma_start(out=out_d[b * P:(b + 1) * P, :], in_=o_sb[:])

            depth = DEPTH if PIPE else 0
            pending = []
            for b in range(B + depth):
                if b < B:
                    pending.append((*compute_block(b), b))
                if len(pending) > depth or (b >= B and pending):
                    scatter_block(*pending.pop(0))

    nc.compile()
    return nc


def _compile(B, K, e_pad):
    if not TABLEFIX:
        return _build_nc(B, K, e_pad)
    # Steer the ACT table-load pass: strip Exp/Ln from every set except
    # natural_log_exp_and_others (which genuinely contains both) so Exp and
    # Ln resolve to ONE set id -> a single ACT_TABLE_LOAD instead of two per
    # block (~2.6us vs ~126us).  Membership edit only -- set ids stay honest.
    from concourse.hw_specs import get_activation_tables

    tabs = get_activation_tables("gen3")
    saved = {k: set(v) for k, v in tabs.items()}
    exp = mybir.ActivationFunctionType.Exp
    ln = mybir.ActivationFunctionType.Ln
    for name, fns in tabs.items():
        if name != "natural_log_exp_and_others":
            fns.discard(exp)
            fns.discard(ln)
    try:
        return _build_nc(B, K, e_pad)
    finally:
        for k, v in tabs.items():
            v.clear()
            v.update(saved[k])


# ------------------------------------------------------------------ entry
def kernel(x, edge_index, edge_ft, W_val, b_val, W_mul, b_mul, _trace=False):
    n_nodes = x.shape[0]
    prep = _preprocess(x, edge_index, edge_ft, W_val, b_val, W_mul, b_mul)
    nc = _compile(prep["B"], prep["K"], prep["e_pad"])

    in_maps = []
    for c in range(N_CORES):
        in_maps.append(
            {
                "xsT": prep["xsT"][c], "xrT": prep["xrT"][c],
                "efT": prep["efT"][c], "SEL": prep["SEL"][c],
                "Wsnd": prep["Wsnd"], "Wrcv": prep["Wrcv"],
                "Wef": prep["Wef"],
            }
        )
    res = run_bass_kernel_spmd(nc, in_maps, list(range(N_CORES)), trace=_trace)
    rows = np.concatenate(
        [np.asarray(res.results[c]["out"]) for c in range(N_CORES)], axis=0
    ).astype(np.float32)
    full = rows[prep["row_of_node"]]
    if _trace:
        return full, res
    return full


# revision 13
# speedup vs baseline: 1.1796x; 1.1796x over previous
"""CGC layer (gated graph conv message passing) on 8 trn2 NeuronCores.

Math (per edge e with sender s, receiver r):
    c    = [x[s], x[r], ef[e]]                  # [320]
    vals = softplus(c @ W_val.T + b_val)        # [128]
    gate = sigmoid (c @ W_mul.T + b_mul)        # [128]
    out[r] += vals * gate                       # segment-sum over receivers

Strategy (edge-parallel, receiver-sharded => no cross-core reduction):
  * Host: LPT-balance nodes into 392 blocks of 128 so every block has
    <= K*128 incident edges with K=16 (vs 18 for the naive contiguous
    partition); shard 49 blocks/core.  Pre-gather x[s]/x[r] rows into
    edge-aligned fp16 streams [128, E_pad]; edge features (+bias row)
    feature-major [65, E_pad]; the one-hot scatter selector is also
    prebuilt on the host ([128, E_pad], 1 col per edge) so no DVE work
    is spent building it on device.
  * Device per chunk of 128 edges: 3 fp16 matmuls (fused [val|mul]
    weights, mul half negated, N=256) accumulate [A|-B] in PSUM; ACT Exp
    (single natural_log_exp table set, forced via table-membership
    steering so Exp/Ln never thrash table loads) gives [t|u]; ACT
    Ln(bias=1) gives vals=softplus(A); DVE/GpSimd compute
    msg = vals/(1+u); PE scatter-adds via psum_out += sel.T @ msg.
  * Software pipelining: the scatter matmuls of block b are emitted after
    the main matmuls of block b+1 so the PE never stalls on the ACT->DVE
    msg chain.
"""

import heapq
import os
import sys

sys.path.insert(0, "/opt/trn_rl_repo")

import ml_dtypes
import numpy as np

from concourse import bacc, bass, mybir, tile
from concourse.bass_utils import run_bass_kernel_spmd

N_CORES = 8
P = 128            # partition / chunk size
G = 4              # chunks per PSUM group
NODE_DIM = 128
EDGE_DIM = 64
F16 = mybir.dt.float16
F32 = mybir.dt.float32
F8 = mybir.dt.float8e4
E4M3 = ml_dtypes.float8_e4m3  # IEEE-style e4m3 (max +-240) == TRN FP8_EXP4

PIPE = os.environ.get("CGC_PIPE", "1") == "1"      # software pipelining
DEPTH = int(os.environ.get("CGC_DEPTH", "1"))      # scatter delay (blocks)
SEL8 = os.environ.get("CGC_SEL8", "1") == "1"      # fp8 one-hot selector
EF8 = os.environ.get("CGC_EF8", "0") == "1"        # fp8 edge features
XSR_Q = os.environ.get("CGC_XSRQ", "gpsimd")       # xs/xr DMA queue engine
TABLEFIX = os.environ.get("CGC_TABLEFIX", "1") == "1"
PPOOL_BUFS = int(os.environ.get("CGC_PPOOL_BUFS", "3"))
GATE_MODE = os.environ.get("CGC_GATE", "fused")    # fused | recip

# Constants from RECIPROCAL_APPROX_FAST: Chebyshev-minimax seed pair over the
# [-4.5,-4] interval that x*bitcast(~x) lands in; one inline NR pass gives
# <=0.18% relative error on 1/(1+u) -- far inside the 2e-2 gate.
_GATE_C0 = -0.23549792
_GATE_C1 = 2.0017324


def _register_fused_gate():
    """Register a custom DVE op computing out = recip(in0 + 1) * in1 in one
    Vector instruction (bitwise-NOT reciprocal seed + one Newton step + the
    final multiply), replacing the 3-instruction add/recip/mult gate chain.
    Additive registration via the documented dve_ops extension point; sha is
    computed locally the same way DveOp.compile() checks it."""
    import concourse.dve_ops as dv
    from concourse.dve_spec import AluOp, Bin, Spec, Src0, Src1, C0, C1, C2, lower
    from concourse.dve_uop import DveOpSpec

    name = "CGC_GATE_FUSED"
    for op in dv.OPS:
        if op.name == name:
            return op
    w = Src0 + C2
    nw = Bin(AluOp.BITWISE_NOT, w, w)
    y0 = nw * C0
    y1 = y0 * (C1 - w * y0)
    body = y1 * Src1

    def _ref(in0, in1, s0, s1, imm2):
        wv = in0.astype(np.float32) + np.float32(imm2)
        nwv = (~wv.view(np.int32)).view(np.float32)
        y0v = nwv * np.float32(s0)
        y1v = y0v * (np.float32(s1) - wv * y0v)
        return (y1v * in1).astype(np.float32)

    spec = Spec(body=body, reference=_ref)
    row = max(dv._SUB_OPCODE_FOR_NAME.values()) + 1
    assert row < 0x20, "no free custom-DVE opcode rows"
    dv._SUB_OPCODE_FOR_NAME[name] = row
    shas = {}
    for ver in ("v3", "v4"):
        uops = lower(spec, ver=ver)
        shas[ver] = DveOpSpec(name=name, opcode=row, uops=uops, rd1_en=True).sha(ver)
    op = dv.DveOp(name, spec, subdim=False, uops_sha=shas)
    dv.OPS.append(op)
    dv.CUSTOM_DVE_SPECS[name] = spec
    return op


# ----------------------------------------------------------------- host prep
def _balance_blocks(deg, n_blocks):
    """LPT bin-pack nodes into n_blocks blocks of <=P nodes, balancing the
    per-block edge counts. Returns blk_of[node], pos_in_blk[node], sums."""
    n = deg.shape[0]
    order = np.argsort(-deg, kind="stable")
    heap = [(0, b) for b in range(n_blocks)]
    heapq.heapify(heap)
    used = np.zeros(n_blocks, dtype=np.int64)
    sums = np.zeros(n_blocks, dtype=np.int64)
    blk_of = np.empty(n, dtype=np.int64)
    pos_in_blk = np.empty(n, dtype=np.int64)
    for nid in order:
        while True:
            _, b = heapq.heappop(heap)
            if used[b] < P:
                break
        blk_of[nid] = b
        pos_in_blk[nid] = used[b]
        used[b] += 1
        sums[b] += deg[nid]
        if used[b] < P:
            heapq.heappush(heap, (sums[b], b))
    return blk_of, pos_in_blk, sums


def _preprocess(x, edge_index, edge_ft, W_val, b_val, W_mul, b_mul):
    n_nodes = x.shape[0]
    snd = np.asarray(edge_index[0], dtype=np.int64)
    rcv = np.asarray(edge_index[1], dtype=np.int64)

    blocks_per_core = int(np.ceil(n_nodes / (N_CORES * P)))  # 49 for 50000
    n_blocks = N_CORES * blocks_per_core
    B = blocks_per_core

    deg = np.bincount(rcv, minlength=n_nodes)
    blk_of, pos_in_blk, sums = _balance_blocks(deg, n_blocks)
    k_chunks = int(np.ceil(sums.max() / P))
    k_chunks = max(G, int(np.ceil(k_chunks / G)) * G)
    K = k_chunks
    e_pad = B * K * P

    # edge -> (core, slot)
    eb = blk_of[rcv]
    eorder = np.argsort(eb, kind="stable")
    eb_s = eb[eorder]
    snd_s = snd[eorder]
    rcv_s = rcv[eorder]
    counts = np.bincount(eb_s, minlength=n_blocks)
    starts = np.zeros(n_blocks + 1, dtype=np.int64)
    np.cumsum(counts, out=starts[1:])
    within = np.arange(len(eb_s), dtype=np.int64) - starts[eb_s]
    core_of = eb_s // B
    slot = (eb_s % B) * (K * P) + within

    x16 = np.asarray(x, dtype=np.float32).astype(np.float16)

    xsT = np.zeros((N_CORES, NODE_DIM, e_pad), dtype=np.float16)
    xrT = np.zeros((N_CORES, NODE_DIM, e_pad), dtype=np.float16)
    xsT[core_of, :, slot] = x16[snd_s]
    xrT[core_of, :, slot] = x16[rcv_s]

    ef_dt = E4M3 if EF8 else np.float16
    ef16 = np.asarray(edge_ft, dtype=np.float32).astype(ef_dt)
    efT = np.zeros((N_CORES, EDGE_DIM + 1, e_pad), dtype=ef_dt)
    efT[core_of, :EDGE_DIM, slot] = ef16[eorder]
    efT[:, EDGE_DIM, :] = ef_dt(1.0)          # bias row

    # host-built one-hot scatter selector: SEL[p, chunk*P + j] = (rloc==j)
    chunk = slot // P
    epos = slot % P
    sel_dt = E4M3 if SEL8 else np.float16
    SEL = np.zeros((N_CORES, P, e_pad), dtype=sel_dt)
    SEL[core_of, epos, chunk * P + pos_in_blk[rcv_s]] = sel_dt(1.0)

    # weights: [val | mul] fused on N; mul half negated so one Exp pass
    # yields [e^A | e^-B].
    Wv = np.asarray(W_val, dtype=np.float32)
    Wm = -np.asarray(W_mul, dtype=np.float32)
    bv = np.asarray(b_val, dtype=np.float32)
    bm = -np.asarray(b_mul, dtype=np.float32)
    Wsnd = np.concatenate([Wv[:, :128].T, Wm[:, :128].T], axis=1).astype(np.float16)
    Wrcv = np.concatenate([Wv[:, 128:256].T, Wm[:, 128:256].T], axis=1).astype(
        np.float16
    )
    Wef = np.concatenate(
        [
            np.concatenate([Wv[:, 256:320].T, Wm[:, 256:320].T], axis=1),
            np.concatenate([bv, bm])[None, :],
        ],
        axis=0,
    ).astype(E4M3 if EF8 else np.float16)

    # output row of node n = blk_of[n]*P + pos_in_blk[n] (blocks core-major)
    row_of_node = blk_of * P + pos_in_blk

    return dict(
        xsT=xsT, xrT=xrT, efT=efT, SEL=SEL,
        Wsnd=Wsnd, Wrcv=Wrcv, Wef=Wef,
        B=B, K=K, e_pad=e_pad, row_of_node=row_of_node,
    )


# ------------------------------------------------------------- device kernel
def _build_nc(B, K, e_pad):
    n_groups = K // G
    nc = bacc.Bacc("TRN2", target_bir_lowering=False, debug=False)

    xsT = nc.dram_tensor("xsT", [NODE_DIM, e_pad], F16, kind="ExternalInput")
    xrT = nc.dram_tensor("xrT", [NODE_DIM, e_pad], F16, kind="ExternalInput")
    EF_DT = F8 if EF8 else F16
    SEL_DT = F8 if SEL8 else F16
    efT = nc.dram_tensor("efT", [EDGE_DIM + 1, e_pad], EF_DT, kind="ExternalInput")
    SEL = nc.dram_tensor("SEL", [P, e_pad], SEL_DT, kind="ExternalInput")
    Wsnd_d = nc.dram_tensor("Wsnd", [NODE_DIM, 256], F16, kind="ExternalInput")
    Wrcv_d = nc.dram_tensor("Wrcv", [NODE_DIM, 256], F16, kind="ExternalInput")
    Wef_d = nc.dram_tensor("Wef", [EDGE_DIM + 1, 256], EF_DT, kind="ExternalInput")
    out_d = nc.dram_tensor("out", [B * P, NODE_DIM], F16, kind="ExternalOutput")

    with tile.TileContext(nc) as tc:
        with (
            tc.tile_pool(name="const", bufs=1) as cpool,
            tc.tile_pool(name="stream", bufs=3 + DEPTH) as spool,
            tc.tile_pool(name="blk", bufs=1 + DEPTH) as bpool,
            tc.tile_pool(name="psumAB", bufs=PPOOL_BUFS, space="PSUM") as ppool,
            tc.tile_pool(name="psumOut", bufs=2, space="PSUM") as opool,
        ):
            w_snd = cpool.tile([NODE_DIM, 256], F16)
            w_rcv = cpool.tile([NODE_DIM, 256], F16)
            w_ef = cpool.tile([EDGE_DIM + 1, 256], EF_DT)
            nc.sync.dma_start(out=w_snd[:], in_=Wsnd_d[:])
            nc.sync.dma_start(out=w_rcv[:], in_=Wrcv_d[:])
            nc.sync.dma_start(out=w_ef[:], in_=Wef_d[:])

            gate_op = _register_fused_gate() if GATE_MODE == "fused" else None

            def compute_block(b):
                off = b * K * P
                xs_b = spool.tile([NODE_DIM, K * P], F16, tag="xs")
                xr_b = spool.tile([NODE_DIM, K * P], F16, tag="xr")
                ef_b = spool.tile([EDGE_DIM + 1, K * P], EF_DT, tag="ef")
                sel = spool.tile([P, K, P], SEL_DT, tag="sel")
                xsr_eng = nc.gpsimd if XSR_Q == "gpsimd" else nc.sync
                xsr_eng.dma_start(out=xs_b[:], in_=xsT[:, off:off + K * P])
                xsr_eng.dma_start(out=xr_b[:], in_=xrT[:, off:off + K * P])
                nc.sync.dma_start(out=ef_b[:], in_=efT[:, off:off + K * P])
                nc.sync.dma_start(out=sel[:], in_=SEL[:, off:off + K * P])

                tu = bpool.tile([P, K, 256], F16, tag="tu")
                for g in range(n_groups):
                    pab = ppool.tile([P, G, 256], F32, tag="ab")
                    for q in range(G):
                        c = g * G + q
                        sl = slice(c * P, (c + 1) * P)
                        nc.tensor.matmul(
                            out=pab[:, q, :], lhsT=xs_b[:, sl], rhs=w_snd[:],
                            start=True, stop=False,
                        )
                        nc.tensor.matmul(
                            out=pab[:, q, :], lhsT=xr_b[:, sl], rhs=w_rcv[:],
                            start=False, stop=False,
                        )
                        nc.tensor.matmul(
                            out=pab[:, q, :], lhsT=ef_b[:, sl], rhs=w_ef[:],
                            start=False, stop=True,
                        )
                    gsl = slice(g * G, (g + 1) * G)
                    nc.scalar.activation(
                        out=tu[:, gsl, :], in_=pab[:],
                        func=mybir.ActivationFunctionType.Exp,
                    )

                # vals = ln(1 + t)   [softplus]; tiles are 2D so the fused
                # gate op sees a 1-free-dim src1 (TTSS shape keeps imm2)
                vals = bpool.tile([P, K * P], F16, tag="vals")
                nc.scalar.activation(
                    out=vals[:], in_=tu[:, :, 0:P],
                    func=mybir.ActivationFunctionType.Ln, bias=1.0,
                )
                # msg = vals / (1 + u)
                msg = bpool.tile([P, K * P], F16, tag="msg")
                if gate_op is not None:
                    nc.vector._custom_dve(
                        gate_op, out=msg[:], in0=tu[:, :, P:256], in1=vals[:],
                        s0=_GATE_C0, s1=_GATE_C1, imm2=1.0,
                    )
                else:
                    w32 = bpool.tile([P, K, P], F32, tag="w32")
                    g32 = bpool.tile([P, K, P], F32, tag="g32")
                    nc.vector.tensor_scalar_add(w32[:], tu[:, :, P:256], 1.0)
                    nc.vector.reciprocal_approx_fast(out=g32[:], in_=w32[:])
                    nc.vector.tensor_tensor(
                        out=msg[:], in0=vals[:], in1=g32[:],
                        op=mybir.AluOpType.mult,
                    )
                return sel, msg

            def scatter_block(sel, msg, b):
                pout = opool.tile([P, P], F32, tag="out")
                for c in range(K):
                    nc.tensor.matmul(
                        out=pout[:], lhsT=sel[:, c, :],
                        rhs=msg[:, c * P:(c + 1) * P],
                        start=(c == 0), stop=(c == K - 1),
                    )
                o_sb = bpool.tile([P, P], F16, tag="osb")
                nc.vector.tensor_copy(out=o_sb[:], in_=pout[:])
                nc.sync.dma_start(out=out_d[b * P:(b + 1) * P, :], in_=o_sb[:])

            depth = DEPTH if PIPE else 0
            pending = []
            for b in range(B + depth):
                if b < B:
                    pending.append((*compute_block(b), b))
                if len(pending) > depth or (b >= B and pending):
                    scatter_block(*pending.pop(0))

    nc.compile()
    return nc


def _compile(B, K, e_pad):
    if not TABLEFIX:
        return _build_nc(B, K, e_pad)
    # Steer the ACT table-load pass: strip Exp/Ln from every set except
    # natural_log_exp_and_others (which genuinely contains both) so Exp and
    # Ln resolve to ONE set id -> a single ACT_TABLE_LOAD instead of two per
    # block (~2.6us vs ~126us).  Membership edit only -- set ids stay honest.
    from concourse.hw_specs import get_activation_tables

    tabs = get_activation_tables("gen3")
    saved = {k: set(v) for k, v in tabs.items()}
    exp = mybir.ActivationFunctionType.Exp
    ln = mybir.ActivationFunctionType.Ln
    for name, fns in tabs.items():
        if name != "natural_log_exp_and_others":
            fns.discard(exp)
            fns.discard(ln)
    try:
        return _build_nc(B, K, e_pad)
    finally:
        for k, v in tabs.items():
            v.clear()
            v.update(saved[k])


# ------------------------------------------------------------------ entry
def kernel(x, edge_index, edge_ft, W_val, b_val, W_mul, b_mul, _trace=False):
    n_nodes = x.shape[0]
    prep = _preprocess(x, edge_index, edge_ft, W_val, b_val, W_mul, b_mul)
    nc = _compile(prep["B"], prep["K"], prep["e_pad"])

    in_maps = []
    for c in range(N_CORES):
        in_maps.append(
            {
                "xsT": prep["xsT"][c], "xrT": prep["xrT"][c],
                "efT": prep["efT"][c], "SEL": prep["SEL"][c],
                "Wsnd": prep["Wsnd"], "Wrcv": prep["Wrcv"],
                "Wef": prep["Wef"],
            }
        )
    res = run_bass_kernel_spmd(nc, in_maps, list(range(N_CORES)), trace=_trace)
    rows = np.concatenate(
        [np.asarray(res.results[c]["out"]) for c in range(N_CORES)], axis=0
    ).astype(np.float32)
    full = rows[prep["row_of_node"]]
    if _trace:
        return full, res
    return full
